# revision 13
# baseline (speedup 1.0000x reference)
"""Video attention (L=2048, D=1024, 16 heads) on 8 Trainium2 NeuronCores.

Sharding: tensor-parallel over heads. Each core owns 2 heads (= 128 of the
1024 channels): Wq/Wk/Wv are split column-wise by head, Wo row-wise; each
core emits a full-shape partial output and the host sums the 8 partials
(the "all-reduce after to_out" done at gather time).

v2 layout/scheduling choices (per-core):
  - everything in SBUF is bf16 (halves DMA + SBUF traffic; rel-err budget
    is 2e-2, bf16 keeps us ~1e-3); PSUM accumulation stays fp32
  - activations transposed [channel, token]; matmuls contract along
    partitions; bf16 runs 1 cycle/row at any moving size
  - V is projected *directly transposed* ([key, vchan]) by swapping the
    stationary/moving operands, killing the PE transposes of v1
  - scores [key j, query i]; softmax denominator via the ones-augmented
    65th column of V in the AV matmul (row 64 of the accumulator)
  - the (scores -> exp -> AV) chain is software-pipelined by one j-tile:
    the PE issues scores(j+1) while ACT runs exp(j), so the PE never
    stalls on the activation engine
  - output epilogue (out-proj + stage copy + DMA) for query tile i is
    deferred a few steps into tile i+1's score loop so it overlaps
  - RoPE rotate_half as a constant 128x128 sign-permutation matmul
"""

import numpy as np

import sys

sys.path.insert(0, "/opt/trn_rl_repo")

import concourse.bacc as bacc
import concourse.bass as bass
import concourse.mybir as mybir
import concourse.tile as tile

F32 = mybir.dt.float32
BF16 = mybir.dt.bfloat16

T, H, W, B, D = 2, 32, 32, 1, 1024
NH = 16
HD = D // NH          # 64
L = T * H * W         # 2048
NCORES = 8
C = D // NCORES       # 128 channels (2 heads) per core
NT = L // 512         # 4 token tiles of 512
KT = D // 128         # 8 contraction tiles for projections
JT = L // 128         # 16 key tiles of 128


def build_program(loop_iters=None):
    nc = bacc.Bacc("TRN2", target_bir_lowering=False, debug=False)

    # xT is host-prearranged to [128, (n k l)] so each token tile loads with
    # one straight 8KB-per-partition DMA; weights likewise [128, (k c)]
    xT = nc.dram_tensor("xT", [128, NT * KT * 512], BF16, kind="ExternalInput")
    wq = nc.dram_tensor("wq", [128, KT * C], BF16, kind="ExternalInput")
    wk = nc.dram_tensor("wk", [128, KT * C], BF16, kind="ExternalInput")
    wv = nc.dram_tensor("wv", [128, KT * C], BF16, kind="ExternalInput")
    wo = nc.dram_tensor("wo", [C, D], BF16, kind="ExternalInput")
    cosT = nc.dram_tensor("cosT", [C, L], BF16, kind="ExternalInput")
    sinT = nc.dram_tensor("sinT", [C, L], BF16, kind="ExternalInput")
    rmat = nc.dram_tensor("rmat", [128, 128], BF16, kind="ExternalInput")
    vones = nc.dram_tensor("vones", [128, JT * 130], BF16, kind="ExternalInput")
    out = nc.dram_tensor("out", [L, D], BF16, kind="ExternalOutput")

    with tile.TileContext(nc) as tc:
        with (
            tc.tile_pool(name="res", bufs=1) as res,
            tc.tile_pool(name="sbw", bufs=4) as sbw,
            tc.tile_pool(name="exp", bufs=6) as expp,
            tc.tile_pool(name="ypool", bufs=2) as ypool,
            tc.tile_pool(name="scl", bufs=4) as sclp,
            tc.tile_pool(name="sc", bufs=2, space="PSUM") as scp,
            tc.tile_pool(name="rotp", bufs=2, space="PSUM") as rotp,
            tc.tile_pool(name="up", bufs=2, space="PSUM") as upp,
        ):
            def emit_body():
                # ---- resident SBUF tensors; DMA order = consumption order
                wqs = res.tile([128, KT * C], BF16, tag="wq")
                wks = res.tile([128, KT * C], BF16, tag="wk")
                nc.sync.dma_start(wqs[:], wq[:])
                nc.sync.dma_start(wks[:], wk[:])
                # x in 4 separate tiles (per 512-token block) so tile n's
                # consumers depend only on tile n's DMA
                xts = [res.tile([128, KT * 512], BF16, tag=f"x{n}", name=f"xt{n}")
                       for n in range(NT)]
                for n in range(NT):
                    nc.sync.dma_start(xts[n][:], xT[:, 4096 * n:4096 * (n + 1)])
                wvs = res.tile([128, KT * C], BF16, tag="wv")
                nc.sync.dma_start(wvs[:], wv[:])
                rms = res.tile([128, 128], BF16, tag="rm")
                nc.sync.dma_start(rms[:], rmat[:])
                coss = res.tile([128, L], BF16, tag="cos")
                sins = res.tile([128, L], BF16, tag="sin")
                nc.sync.dma_start(coss[:], cosT[:])
                nc.sync.dma_start(sins[:], sinT[:])
                # v in [key, vchan] layout, 65-wide per head (65th col = 1.0
                # supplies the softmax denominator through the AV matmul)
                vaug = res.tile([128, JT * 130], BF16, tag="vaug")
                nc.sync.dma_start(vaug[:], vones[:])
                wos = res.tile([128, D], BF16, tag="wo")
                nc.sync.dma_start(wos[:], wo[:])

                qns = res.tile([128, L], BF16, tag="qn")
                kns = res.tile([128, L], BF16, tag="kn")

                # ---- phase A block: projections + rope (Q/K) and
                # direct-transposed V for one 512-token tile
                def emit_a(n):
                    nsl = slice(512 * n, 512 * (n + 1))
                    xt = xts[n]
                    ps = scp.tile([128, 1024], F32, tag="sc", name=f"pqk{n}")
                    for wsb, half in ((wqs, 0), (wks, 1)):
                        for kk in range(KT):
                            nc.tensor.matmul(
                                ps[:, 512 * half:512 * (half + 1)],
                                wsb[:, C * kk:C * (kk + 1)],
                                xt[:, 512 * kk:512 * (kk + 1)],
                                start=(kk == 0),
                                stop=(kk == KT - 1),
                            )
                    # direct-transposed V for this token tile's 4 key tiles;
                    # also fills the PE while ACT/DVE copy craw out of PSUM
                    vps = rotp.tile([128, 512], F32, tag="rot", name=f"v{n}")
                    for jj in range(4):
                        for kk in range(KT):
                            nc.tensor.matmul(
                                vps[:, 128 * jj:128 * (jj + 1)],
                                xt[:, 512 * kk + 128 * jj:512 * kk + 128 * (jj + 1)],
                                wvs[:, C * kk:C * (kk + 1)],
                                start=(kk == 0),
                                stop=(kk == KT - 1),
                            )
                    cq = sbw.tile([128, 512], BF16, tag="cq")
                    ck = sbw.tile([128, 512], BF16, tag="ck")
                    nc.scalar.copy(cq[:], ps[:, 0:512])
                    nc.vector.tensor_copy(ck[:], ps[:, 512:1024])
                    for craw, dest in ((cq, qns), (ck, kns)):
                        rot = rotp.tile([128, 512], F32, tag="rot")
                        nc.tensor.matmul(rot[:], rms[:], craw[:], start=True, stop=True)
                        qc = sbw.tile([128, 512], BF16, tag="qc")
                        nc.vector.tensor_mul(qc[:], craw[:], coss[:, nsl])
                        t2 = sbw.tile([128, 512], BF16, tag="t2")
                        nc.vector.tensor_mul(t2[:], rot[:], sins[:, nsl])
                        nc.vector.tensor_add(dest[:, nsl], qc[:], t2[:])
                    for jj in range(4):
                        j = 4 * n + jj
                        nc.vector.tensor_copy(
                            vaug[:, 130 * j:130 * j + 64],
                            vps[:, 128 * jj:128 * jj + 64])
                        nc.vector.tensor_copy(
                            vaug[:, 130 * j + 65:130 * j + 129],
                            vps[:, 128 * jj + 64:128 * jj + 128])

                # ---- phase B: attention, software-pipelined by one j-tile
                u = {}
                exps = {}
                pending = []  # deferred epilogues: (emit_at_idx, i)

                def emit_scores(i, j):
                    isl = slice(512 * i, 512 * (i + 1))
                    sps = scp.tile([128, 1024], F32, tag="sc", name=f"s{i}_{j}")
                    for h in range(2):
                        hp = slice(64 * h, 64 * (h + 1))
                        nc.tensor.matmul(
                            sps[:, 512 * h:512 * (h + 1)],
                            kns[hp, 128 * j:128 * (j + 1)],
                            qns[hp, isl],
                            start=True,
                            stop=True,
                        )
                    e = expp.tile([128, 1024], BF16, tag="e")
                    nc.scalar.activation(e[:], sps[:], mybir.ActivationFunctionType.Exp)
                    exps[(i, j)] = e

                def emit_av(i, j):
                    e = exps.pop((i, j))
                    for h in range(2):
                        nc.tensor.matmul(
                            u[(i, h)][:],
                            vaug[:, 130 * j + 65 * h:130 * j + 65 * (h + 1)],
                            e[:, 512 * h:512 * (h + 1)],
                            start=(j == 0),
                            stop=(j == JT - 1),
                        )

                def emit_norm(i):
                    # u -> y = u / sumexp  (frees the u PSUM tiles)
                    y = ypool.tile([128, 512], BF16, tag="y", name=f"y{i}")
                    for h in range(2):
                        rec = sclp.tile([1, 512], F32, tag="rec")
                        nc.vector.reciprocal(rec[:], u[(i, h)][64:65, :])
                        scl = sclp.tile([64, 512], F32, tag="scl")
                        nc.gpsimd.partition_broadcast(scl[:], rec[:])
                        nc.vector.tensor_mul(
                            y[64 * h:64 * (h + 1), :], u[(i, h)][0:64, :], scl[:])
                        del u[(i, h)]
                    return y

                def emit_outproj(i, y):
                    stage = ypool.tile([128, 4096], BF16, tag="stage", name=f"st{i}")
                    for m in range(4):
                        for n2 in range(2):
                            ops_ = rotp.tile([128, 512], F32, tag="rot")
                            nc.tensor.matmul(
                                ops_[:],
                                y[:, 128 * m:128 * (m + 1)],
                                wos[:, 512 * n2:512 * (n2 + 1)],
                                start=True,
                                stop=True,
                            )
                            nc.vector.tensor_copy(
                                stage[:, 1024 * m + 512 * n2:1024 * m + 512 * (n2 + 1)],
                                ops_[:],
                            )
                    nc.sync.dma_start(
                        out[512 * i:512 * (i + 1), :].rearrange(
                            "(m p) d -> p m d", p=128),
                        stage[:].rearrange("p (m d) -> p m d", m=4),
                    )

                sched = [("A", n) for n in range(NT)]
                sched += [("P", (i, j)) for i in range(NT) for j in range(JT)]

                ys = {}
                last_pair = None
                idx = 0
                for kind, arg in sched:
                    if kind == "A":
                        emit_a(arg)
                        continue
                    i, j = arg
                    if j == 0:
                        u[(i, 0)] = upp.tile([65, 512], F32, tag="u", name=f"u{i}_0")
                        u[(i, 1)] = upp.tile([65, 512], F32, tag="u", name=f"u{i}_1")
                    emit_scores(i, j)
                    if last_pair is not None:
                        pi, pj = last_pair
                        emit_av(pi, pj)
                        if pj == JT - 1:
                            ys[pi] = emit_norm(pi)
                            pending.append((idx + 6, pi))
                    last_pair = (i, j)
                    while pending and pending[0][0] <= idx:
                        _, ei = pending.pop(0)
                        emit_outproj(ei, ys.pop(ei))
                    idx += 1
                emit_av(*last_pair)
                ys[NT - 1] = emit_norm(NT - 1)
                for _, ei in pending:
                    emit_outproj(ei, ys.pop(ei))
                emit_outproj(NT - 1, ys.pop(NT - 1))

            if loop_iters is None:
                emit_body()
            else:
                with tc.For_i(0, loop_iters, 1):
                    emit_body()

    nc.compile()
    return nc


_NC = None


def _get_nc():
    global _NC
    if _NC is None:
        _NC = build_program()
    return _NC


def make_in_maps(x, rope_emb_L_1_1_D, Wq, Wk, Wv, Wo):
    """Host-side prep: shard weights by head, transpose x, build rope tables.
    Everything shipped to the device as bf16."""
    import ml_dtypes

    bf16 = ml_dtypes.bfloat16
    x = np.asarray(x, dtype=np.float32)
    rope = np.asarray(rope_emb_L_1_1_D, dtype=np.float32).reshape(L, HD)
    Wq = np.asarray(Wq, dtype=np.float32)
    Wk = np.asarray(Wk, dtype=np.float32)
    Wv = np.asarray(Wv, dtype=np.float32)
    Wo = np.asarray(Wo, dtype=np.float32)

    xs_flat = x.reshape(L, D)  # B == 1
    # [128, (n k l)]: element (p, n, k, l) = x[512n+l, 128k+p]
    xT = np.ascontiguousarray(
        xs_flat.reshape(NT, 512, KT, 128).transpose(3, 0, 2, 1).reshape(
            128, NT * KT * 512)).astype(bf16)

    def wprep(wt):  # [D, C] -> [128, (k c)]: (p, k, c) = wt[128k+p, c]
        return np.ascontiguousarray(
            wt.reshape(KT, 128, C).transpose(1, 0, 2).reshape(128, KT * C)
        ).astype(bf16)

    cos = np.cos(rope).T  # [HD, L]
    sin = np.sin(rope).T
    cosT = np.concatenate([cos, cos], axis=0).astype(bf16)  # [128, L]
    sinT = np.concatenate([sin, sin], axis=0).astype(bf16)

    # rot(q)[d'] = sum_k rmat[k, d'] q[k]; per 64-block: first 32 rows get
    # -q[d+32], last 32 get +q[d-32]  (signs folded in so sinT is plain sin)
    rmat = np.zeros((128, 128), dtype=np.float32)
    for b in (0, 64):
        for m in range(32):
            rmat[b + m + 32, b + m] = -1.0
        for m in range(32, 64):
            rmat[b + m - 32, b + m] = 1.0
    rmat = rmat.astype(bf16)

    scale = HD ** -0.5
    vones = np.ones((128, JT * 130), dtype=bf16)
    in_maps = []
    for c in range(NCORES):
        rows = slice(C * c, C * (c + 1))
        in_maps.append({
            "xT": xT,
            "wq": wprep((scale * Wq[rows, :]).T),
            "wk": wprep(Wk[rows, :].T),
            "wv": wprep(Wv[rows, :].T),
            "wo": np.ascontiguousarray(Wo[:, rows].T).astype(bf16),
            "cosT": cosT,
            "sinT": sinT,
            "rmat": rmat,
            "vones": vones,
        })
    return in_maps


class _Runner:
    """Persistent jitted SPMD executable (mirrors bass2jax.run_bass_via_pjrt
    but caches the compiled callable, and builds the donated output buffers
    on-device instead of shipping zeros through the tunnel)."""

    def __init__(self, nc):
        import jax
        import jax.numpy as jnp
        from jax.sharding import Mesh, PartitionSpec, NamedSharding
        from jax.experimental.shard_map import shard_map
        from concourse import bass2jax

        bass2jax.install_neuronx_cc_hook()
        self.jax = jax
        self.nc = nc
        part_name = nc.partition_id_tensor.name if nc.partition_id_tensor else None
        in_names, out_names, out_avals, zero_shapes = [], [], [], []
        for alloc in nc.m.functions[0].allocations:
            if not isinstance(alloc, mybir.MemoryLocationSet):
                continue
            name = alloc.memorylocations[0].name
            if alloc.kind == "ExternalInput":
                if name != part_name:
                    in_names.append(name)
            elif alloc.kind == "ExternalOutput":
                out_names.append(name)
                shape = tuple(alloc.tensor_shape)
                dtype = mybir.dt.np(alloc.dtype)
                out_avals.append(jax.core.ShapedArray(shape, dtype))
                zero_shapes.append((shape, dtype))
        self.in_names = list(in_names)
        self.out_names = list(out_names)
        self.out_avals = out_avals
        self.zero_shapes = zero_shapes
        n_params = len(in_names)
        n_outs = len(out_names)
        all_in_names = in_names + out_names
        if part_name is not None:
            all_in_names = all_in_names + [part_name]

        def _body(*args):
            operands = list(args)
            if part_name is not None:
                operands.append(bass2jax.partition_id_tensor())
            outs = bass2jax._bass_exec_p.bind(
                *operands,
                out_avals=tuple(out_avals),
                in_names=tuple(all_in_names),
                out_names=tuple(out_names),
                lowering_input_output_aliases=(),
                sim_require_finite=True,
                sim_require_nnan=True,
                nc=nc,
            )
            return tuple(outs)

        devices = jax.devices()[:NCORES]
        self.mesh = Mesh(np.asarray(devices), ("core",))
        self.pspec = PartitionSpec("core")
        self.sh = NamedSharding(self.mesh, self.pspec)
        in_specs = (self.pspec,) * (n_params + n_outs)
        out_specs = (self.pspec,) * n_outs
        self.sharded = jax.jit(
            shard_map(_body, mesh=self.mesh, in_specs=in_specs,
                      out_specs=out_specs, check_rep=False),
            donate_argnums=tuple(range(n_params, n_params + n_outs)),
            keep_unused=True,
        )
        # donated output buffers built on-device (fresh ones per call)
        self._zeros_fn = jax.jit(
            lambda: tuple(
                jnp.zeros((NCORES * s[0], *s[1:]), dt) for s, dt in zero_shapes
            ),
            out_shardings=tuple(self.sh for _ in zero_shapes),
        )

    def concat_inputs(self, in_maps):
        return [
            np.concatenate([np.asarray(m[name]) for m in in_maps], axis=0)
            for name in self.in_names
        ]

    def device_inputs(self, in_maps):
        return [self.jax.device_put(a, self.sh) for a in self.concat_inputs(in_maps)]

    def fresh_zeros(self):
        return list(self._zeros_fn())

    def __call__(self, dev_in, zeros):
        outs = self.sharded(*dev_in, *zeros)
        self.jax.block_until_ready(outs)
        return outs

    def run_np(self, in_maps):
        outs = self(self.device_inputs(in_maps), self.fresh_zeros())
        per_core = []
        for c in range(NCORES):
            d = {}
            for idx, name in enumerate(self.out_names):
                shape = self.out_avals[idx].shape
                d[name] = np.asarray(outs[idx]).reshape(NCORES, *shape)[c]
            per_core.append(d)
        return per_core


_RUNNER = None


def _get_runner():
    global _RUNNER
    if _RUNNER is None:
        _RUNNER = _Runner(_get_nc())
    return _RUNNER


def run(inputs):
    runner = _get_runner()
    in_maps = make_in_maps(**inputs)
    results = runner.run_np(in_maps)
    partial = np.zeros((L, D), dtype=np.float32)
    for r in results:
        partial += r["out"].astype(np.float32)
    return partial.reshape(T, H, W, B, D)


def kernel(**inputs):
    return run(inputs)
